# revision 4
# baseline (speedup 1.0000x reference)
"""GIN message-passing network on 8 Trainium2 NeuronCores.

Strategy (data parallel over nodes/edges, dst-sharded):
 - Nodes are split into 8 contiguous chunks (one per core), padded to a
   multiple of 128.  Edges are assigned to the core that owns their dst.
 - Per layer, using linearity of the aggregation:
       h = MLP(z + A z) ;  u = z @ W1 ;  m = relu(u + A u + b1)
   so each core computes its local u chunk, all-gathers u (row-major
   table in DRAM), then gathers u[src] for its edges with dma_gather and
   segment-sums them into per-128-node windows with one small matmul per
   128-edge chunk (selection matrix built on the vector engine from a
   precomputed window-local dst table).  BatchNorm statistics are
   all-reduced ([64,2] sums).  Activations live feature-on-partition.
 - Gather indices are int16, so the u table is addressed through two
   base offsets (rows < 32768 and >= 32768); edges are split into two
   streams accordingly, each with its own chunk schedule per window.
"""

import math
import numpy as np

NCORES = 8
P = 128


def _ceil_div(a, b):
    return -(-a // b)


def _pack_idx(idx):
    """Wrap an int array into the [128, n/16] int16 layout dma_gather wants
    (index i at [i % 16, i // 16], replicated across the 8 Q7 stripes)."""
    n = len(idx)
    assert n % 16 == 0
    arr = np.asarray(idx, dtype=np.int16).reshape(-1, 16).T.copy()  # [16, n/16]
    return np.tile(arr, (8, 1))


class Cfg:
    pass


def make_config(N, E, G, DIN, H, L, edge_index, batch, split=32768,
                call_chunks=32, reps=1):
    cfg = Cfg()
    cfg.N, cfg.E, cfg.G, cfg.DIN, cfg.H, cfg.L = N, E, G, DIN, H, L
    assert N % NCORES == 0
    cfg.nloc = N // NCORES
    cfg.nlocp = _ceil_div(cfg.nloc, P) * P
    cfg.nw = cfg.nlocp // P
    cfg.ntab = NCORES * cfg.nlocp          # padded u-table rows
    cfg.split = split
    assert cfg.split % 2 == 0 and cfg.split <= 32768
    assert cfg.ntab - cfg.split < 32768 and cfg.split < cfg.ntab
    cfg.call_chunks = call_chunks
    cfg.reps = reps
    cfg.eps = 1e-5
    cfg.n512 = _ceil_div(cfg.nlocp, 512)
    assert cfg.nlocp % 512 == 0 or cfg.nlocp % 128 == 0

    src = np.asarray(edge_index[0], dtype=np.int64)
    dst = np.asarray(edge_index[1], dtype=np.int64)
    owner = dst // cfg.nloc
    ld = dst - owner * cfg.nloc
    src_pad = (src // cfg.nloc) * cfg.nlocp + (src % cfg.nloc)

    # per-core edge lists sorted by local dst
    per_core = []
    for c in range(NCORES):
        m = owner == c
        sp, l = src_pad[m], ld[m]
        o = np.argsort(l, kind="stable")
        per_core.append((sp[o], l[o]))

    # chunk schedule: CA[w], CB[w] = shared (max over cores) chunk counts
    CA = np.zeros(cfg.nw, np.int64)
    CB = np.zeros(cfg.nw, np.int64)
    counts = []  # per core: (nA[w], nB[w])
    for c in range(NCORES):
        sp, l = per_core[c]
        w = l >> 7
        isb = sp >= cfg.split
        nA = np.bincount(w[~isb], minlength=cfg.nw)
        nB = np.bincount(w[isb], minlength=cfg.nw)
        counts.append((nA, nB))
        CA = np.maximum(CA, _ceil_div(nA, P))
        CB = np.maximum(CB, _ceil_div(nB, P))
    # ensure every window has at least one chunk so psum gets initialized
    for w in range(cfg.nw):
        if CA[w] == 0 and CB[w] == 0:
            CA[w] = 1
    cfg.CA, cfg.CB = CA, CB
    cfg.nchA, cfg.nchB = int(CA.sum()), int(CB.sum())
    cfg.nch = cfg.nchA + cfg.nchB
    # column index of first chunk of each window in each stream
    cfg.acol = np.concatenate([[0], np.cumsum(CA)])[:-1]
    cfg.bcol = np.concatenate([[0], np.cumsum(CB)])[:-1]

    # per-core data: gather index streams + d-table
    cfg.gidxA, cfg.gidxB, cfg.dtab = [], [], []
    for c in range(NCORES):
        sp, l = per_core[c]
        w = l >> 7
        isb = sp >= cfg.split
        dloc = (l & 127).astype(np.float32)
        iA = np.zeros(cfg.nchA * P, np.int64)
        iB = np.zeros(cfg.nchB * P, np.int64)
        dt = np.full((P, cfg.nch), -1.0, np.float32)
        for wi in range(cfg.nw):
            mA = (w == wi) & ~isb
            mB = (w == wi) & isb
            for (mask, C, col0, stream, base) in (
                (mA, CA, cfg.acol, iA, 0),
                (mB, CB, cfg.bcol, iB, cfg.split),
            ):
                vals = sp[mask] - base
                ds = dloc[mask]
                n = len(vals)
                off = col0[wi] * P
                stream[off:off + n] = vals
                # d-table column block for this window's chunks
                colbase = col0[wi] + (0 if base == 0 else cfg.nchA)
                nchunks = C[wi]
                dcol = np.full(nchunks * P, -1.0, np.float32)
                dcol[:n] = ds
                dt[:, colbase:colbase + nchunks] = dcol.reshape(nchunks, P).T
        cfg.gidxA.append(_pack_idx(iA))
        cfg.gidxB.append(_pack_idx(iB))
        cfg.dtab.append(dt)

    # pooling: slots per core (fixed across cores)
    b = np.asarray(batch, dtype=np.int64)
    cfg.g_lo = []
    spans = []
    for c in range(NCORES):
        bc = b[c * cfg.nloc:(c + 1) * cfg.nloc]
        cfg.g_lo.append(int(bc[0]))
        spans.append(int(bc[-1]) - int(bc[0]) + 1)
    cfg.nslot = _ceil_div(max(spans), P) * P
    assert cfg.nslot <= 128, f"graph span {max(spans)} exceeds 128 slots"
    cfg.nslot = 128
    cfg.ptab = []
    for c in range(NCORES):
        bc = b[c * cfg.nloc:(c + 1) * cfg.nloc]
        pt = np.zeros((P, cfg.nw * P), np.float32)
        rows = np.arange(cfg.nloc)
        slot = bc - cfg.g_lo[c]
        pt[rows % P, (rows // P) * P + slot] = 1.0
        cfg.ptab.append(pt)
    return cfg


def build_program(cfg):
    import concourse.bacc as bacc
    import concourse.tile as tile
    from concourse import mybir

    f32 = mybir.dt.float32
    i16 = mybir.dt.int16
    Alu = mybir.AluOpType
    Act = mybir.ActivationFunctionType

    H, DIN, L = cfg.H, cfg.DIN, cfg.L
    nlocp, nw, n512 = cfg.nlocp, cfg.nw, cfg.n512
    nloc = cfg.nloc

    nc = bacc.Bacc()

    # ---- parameters -------------------------------------------------
    xt_p = nc.declare_dram_parameter("xt", [DIN, nlocp], f32, isOutput=False)
    w1_p = [nc.declare_dram_parameter(f"w1_{l}", [DIN if l == 0 else H, H], f32,
                                      isOutput=False) for l in range(L)]
    w2_p = [nc.declare_dram_parameter(f"w2_{l}", [H, H], f32, isOutput=False)
            for l in range(L)]
    vecs_p = nc.declare_dram_parameter("vecs", [H, 4 * L], f32, isOutput=False)
    idxA_p = nc.declare_dram_parameter("gidxA", [P, cfg.nchA * P // 16], i16,
                                       isOutput=False)
    idxB_p = nc.declare_dram_parameter("gidxB", [P, max(cfg.nchB, 1) * P // 16], i16,
                                       isOutput=False)
    dtab_p = nc.declare_dram_parameter("dtab", [P, cfg.nch], f32, isOutput=False)
    ptab_p = nc.declare_dram_parameter("ptab", [P, nw * P], f32, isOutput=False)
    iota_p = nc.declare_dram_parameter("iota", [P, P], f32, isOutput=False)
    ident_p = nc.declare_dram_parameter("ident", [H, H], f32, isOutput=False)
    z_out = nc.declare_dram_parameter("z_out", [L, nlocp, H], f32, isOutput=True)
    g_out = nc.declare_dram_parameter("g_out", [L, P, H], f32, isOutput=True)

    # ---- internal DRAM (collectives) --------------------------------
    ag_in = nc.dram_tensor("ag_in", [nlocp, H], f32)
    u_full = nc.dram_tensor("u_full", [cfg.ntab, H], f32, addr_space="Shared")
    ar_in = nc.dram_tensor("ar_in", [H, 2], f32)
    ar_out = nc.dram_tensor("ar_out", [H, 2], f32, addr_space="Shared")
    rg = [list(range(NCORES))]

    # gather calls: (start_chunk, n_chunks) per call, per stream
    def calls_of(nch):
        out = []
        s = 0
        while s < nch:
            n = min(cfg.call_chunks, nch - s)
            out.append((s, n))
            s += n
        return out

    callsA = calls_of(cfg.nchA)
    callsB = calls_of(cfg.nchB)

    # map chunk -> (call_index, slot)
    def chunk_map(calls):
        m = {}
        for ci, (s, n) in enumerate(calls):
            for k in range(n):
                m[s + k] = (ci, k)
        return m

    mapA = chunk_map(callsA)
    mapB = chunk_map(callsB)

    # stats chunks covering exactly nloc columns
    stat_chunks = []
    off = 0
    while off < nloc:
        n = min(512, nloc - off)
        stat_chunks.append((off, n))
        off += n
    nstat = len(stat_chunks)

    with tile.TileContext(nc) as tc:
        with (
            tc.tile_pool(name="const", bufs=1) as constp,
            tc.tile_pool(name="act", bufs=1) as actp,
            tc.tile_pool(name="uown", bufs=1) as uownp,
            tc.tile_pool(name="ga", bufs=3) as gap,
            tc.tile_pool(name="gb", bufs=2) as gbp,
            tc.tile_pool(name="sp", bufs=4) as spp,
            tc.tile_pool(name="small", bufs=4) as smallp,
            tc.tile_pool(name="urow", bufs=3) as urowp,
            tc.tile_pool(name="zrow", bufs=3) as zrowp,
            tc.tile_pool(name="psA", bufs=2, space="PSUM") as psA,
            tc.tile_pool(name="psB", bufs=2, space="PSUM") as psB,
            tc.tile_pool(name="psUH", bufs=2, space="PSUM") as psUH,
            tc.tile_pool(name="ps128", bufs=1, space="PSUM") as ps128,
            tc.tile_pool(name="psPool", bufs=1, space="PSUM") as psPool,
        ):
            # resident constants
            idxA_t = constp.tile([P, cfg.nchA * P // 16], i16)
            nc.sync.dma_start(out=idxA_t[:], in_=idxA_p[:, :])
            idxB_t = constp.tile([P, max(cfg.nchB, 1) * P // 16], i16)
            nc.sync.dma_start(out=idxB_t[:], in_=idxB_p[:, :])
            dtab_t = constp.tile([P, cfg.nch], f32)
            nc.sync.dma_start(out=dtab_t[:], in_=dtab_p[:, :])
            ptab_t = constp.tile([P, nw * P], f32)
            nc.sync.dma_start(out=ptab_t[:], in_=ptab_p[:, :])
            iota_t = constp.tile([P, P], f32)
            nc.sync.dma_start(out=iota_t[:], in_=iota_p[:, :])
            ident_t = constp.tile([H, H], f32)
            nc.sync.dma_start(out=ident_t[:], in_=ident_p[:, :])
            w1_t = []
            w2_t = []
            for l in range(L):
                d = DIN if l == 0 else H
                t = constp.tile([d, H], f32, tag=f"w1_{l}")
                nc.sync.dma_start(out=t[:], in_=w1_p[l][:, :])
                w1_t.append(t)
                t = constp.tile([H, H], f32, tag=f"w2_{l}")
                nc.sync.dma_start(out=t[:], in_=w2_p[l][:, :])
                w2_t.append(t)
            vecs_t = constp.tile([H, 4 * L], f32)
            nc.sync.dma_start(out=vecs_t[:], in_=vecs_p[:, :])
            eps_t = constp.tile([H, 1], f32)
            nc.vector.memset(eps_t[:], cfg.eps)

            for rep in range(cfg.reps):
                xt_t = actp.tile([DIN, nlocp], f32, tag="act")
                nc.sync.dma_start(out=xt_t[:], in_=xt_p[:, :])
                act_t = xt_t

                for l in range(L):
                    d_in = DIN if l == 0 else H
                    b1 = vecs_t[:, 4 * l + 0:4 * l + 1]
                    b2 = vecs_t[:, 4 * l + 1:4 * l + 2]
                    gam = vecs_t[:, 4 * l + 2:4 * l + 3]
                    bet = vecs_t[:, 4 * l + 3:4 * l + 4]

                    # ---- u = act @ W1 : transposed copy + row-major to DRAM
                    uown_t = uownp.tile([H, nlocp], f32, tag="uown")
                    for j in range(n512):
                        c0 = j * 512
                        cn = min(512, nlocp - c0)
                        pu = psUH.tile([H, 512], f32, tag="psuh")
                        nc.tensor.matmul(out=pu[:, :cn], lhsT=w1_t[l][:],
                                         rhs=act_t[:, c0:c0 + cn],
                                         start=True, stop=True)
                        nc.scalar.copy(out=uown_t[:, c0:c0 + cn], in_=pu[:, :cn])
                    for t in range(nw):
                        c0 = t * P
                        pur = ps128.tile([P, H], f32, tag="p128")
                        nc.tensor.matmul(out=pur[:], lhsT=act_t[:, c0:c0 + P],
                                         rhs=w1_t[l][:], start=True, stop=True)
                        ur = urowp.tile([P, H], f32, tag="urow")
                        nc.scalar.copy(out=ur[:], in_=pur[:])
                        nc.sync.dma_start(out=ag_in[c0:c0 + P, :], in_=ur[:])

                    # ---- all-gather u
                    nc.gpsimd.collective_compute(
                        "AllGather", mybir.AluOpType.bypass,
                        ins=[ag_in[:, :]], outs=[u_full[:, :]],
                        replica_groups=rg,
                    )

                    # ---- gathers
                    gtilesA = []
                    for (s, n) in callsA:
                        g = gap.tile([P, cfg.call_chunks, H], f32, tag="ga")
                        nc.gpsimd.dma_gather(
                            out_ap=g[:, :n, :],
                            in_ap=u_full[0:cfg.split, :],
                            idxs_ap=idxA_t[:, s * 8:(s + n) * 8],
                            num_idxs=n * P, num_idxs_reg=n * P,
                            elem_size=H, single_packet=False,
                        )
                        gtilesA.append(g)
                    gtilesB = []
                    for (s, n) in callsB:
                        g = gbp.tile([P, cfg.call_chunks, H], f32, tag="gb")
                        nc.gpsimd.dma_gather(
                            out_ap=g[:, :n, :],
                            in_ap=u_full[cfg.split:cfg.ntab, :],
                            idxs_ap=idxB_t[:, s * 8:(s + n) * 8],
                            num_idxs=n * P, num_idxs_reg=n * P,
                            elem_size=H, single_packet=False,
                        )
                        gtilesB.append(g)

                    # ---- windows: selection-matrix matmuls into PSUM,
                    # accumulated in place into uown (becomes m_pre -> m -> r)
                    mpre_t = uown_t
                    for w in range(nw):
                        pa = None
                        if cfg.CA[w] > 0:
                            pa = psA.tile([H, P], f32, tag="psa")
                            for k in range(cfg.CA[w]):
                                ch = int(cfg.acol[w]) + k
                                ci, slot = mapA[ch]
                                st = spp.tile([P, P], f32, tag="s")
                                nc.vector.tensor_scalar(
                                    out=st[:], in0=iota_t[:],
                                    scalar1=dtab_t[:, ch:ch + 1],
                                    scalar2=None, op0=Alu.is_equal)
                                nc.tensor.matmul(
                                    out=pa[:], lhsT=gtilesA[ci][:, slot, :],
                                    rhs=st[:], start=(k == 0),
                                    stop=(k == cfg.CA[w] - 1))
                        pb = None
                        if cfg.CB[w] > 0:
                            pb = psB.tile([H, P], f32, tag="psb")
                            for k in range(cfg.CB[w]):
                                ch = int(cfg.bcol[w]) + k
                                ci, slot = mapB[ch]
                                st = spp.tile([P, P], f32, tag="s")
                                nc.vector.tensor_scalar(
                                    out=st[:], in0=iota_t[:],
                                    scalar1=dtab_t[:, cfg.nchA + ch:cfg.nchA + ch + 1],
                                    scalar2=None, op0=Alu.is_equal)
                                nc.tensor.matmul(
                                    out=pb[:], lhsT=gtilesB[ci][:, slot, :],
                                    rhs=st[:], start=(k == 0),
                                    stop=(k == cfg.CB[w] - 1))
                        c0 = w * P
                        first = pa if pa is not None else pb
                        nc.vector.tensor_add(out=mpre_t[:, c0:c0 + P],
                                             in0=first[:],
                                             in1=uown_t[:, c0:c0 + P])
                        if pa is not None and pb is not None:
                            nc.vector.tensor_add(out=mpre_t[:, c0:c0 + P],
                                                 in0=mpre_t[:, c0:c0 + P],
                                                 in1=pb[:])

                    # ---- m = relu(mpre + b1) in place ; h = m @ W2 ;
                    # r = relu(h + b2) overwrites m tile by tile
                    m_t = mpre_t
                    for j in range(n512):
                        c0 = j * 512
                        cn = min(512, nlocp - c0)
                        nc.scalar.activation(out=m_t[:, c0:c0 + cn],
                                             in_=m_t[:, c0:c0 + cn],
                                             func=Act.Relu, bias=b1, scale=1.0)
                    r_t = m_t
                    for j in range(n512):
                        c0 = j * 512
                        cn = min(512, nlocp - c0)
                        ph = psUH.tile([H, 512], f32, tag="psuh")
                        nc.tensor.matmul(out=ph[:, :cn], lhsT=w2_t[l][:],
                                         rhs=m_t[:, c0:c0 + cn],
                                         start=True, stop=True)
                        nc.scalar.activation(out=r_t[:, c0:c0 + cn],
                                             in_=ph[:, :cn],
                                             func=Act.Relu, bias=b2, scale=1.0)

                    # ---- batchnorm stats over the real nloc columns
                    stats_t = smallp.tile([H, nstat, 6], f32, tag="stats")
                    for j, (o, n) in enumerate(stat_chunks):
                        nc.vector.bn_stats(out=stats_t[:, j, :],
                                           in_=r_t[:, o:o + n])
                    mv = smallp.tile([H, 2], f32, tag="mv")
                    nc.vector.bn_aggr(out=mv[:], in_=stats_t[:])
                    # sums: sx = mean*nloc ; sxx = (var + mean^2)*nloc
                    sums = smallp.tile([H, 2], f32, tag="sums")
                    nc.vector.tensor_scalar(out=sums[:, 0:1], in0=mv[:, 0:1],
                                            scalar1=float(nloc), scalar2=None,
                                            op0=Alu.mult)
                    sq = smallp.tile([H, 1], f32, tag="sq")
                    nc.vector.tensor_tensor(out=sq[:], in0=mv[:, 0:1],
                                            in1=mv[:, 0:1], op=Alu.mult)
                    nc.vector.tensor_add(out=sq[:], in0=sq[:], in1=mv[:, 1:2])
                    nc.vector.tensor_scalar(out=sums[:, 1:2], in0=sq[:],
                                            scalar1=float(nloc), scalar2=None,
                                            op0=Alu.mult)
                    nc.gpsimd.dma_start(out=ar_in[:, :], in_=sums[:])
                    nc.gpsimd.collective_compute(
                        "AllReduce", mybir.AluOpType.add,
                        ins=[ar_in[:, :]], outs=[ar_out[:, :]],
                        replica_groups=rg,
                    )
                    gs = smallp.tile([H, 2], f32, tag="gs")
                    nc.gpsimd.dma_start(out=gs[:], in_=ar_out[:, :])
                    # mu = gs[:,0]/N ; ex2 = gs[:,1]/N ; var = ex2 - mu^2
                    mu = smallp.tile([H, 1], f32, tag="mu")
                    nc.vector.tensor_scalar(out=mu[:], in0=gs[:, 0:1],
                                            scalar1=1.0 / cfg.N, scalar2=None,
                                            op0=Alu.mult)
                    var = smallp.tile([H, 1], f32, tag="var")
                    nc.vector.tensor_scalar(out=var[:], in0=gs[:, 1:2],
                                            scalar1=1.0 / cfg.N, scalar2=None,
                                            op0=Alu.mult)
                    musq = smallp.tile([H, 1], f32, tag="musq")
                    nc.vector.tensor_tensor(out=musq[:], in0=mu[:], in1=mu[:],
                                            op=Alu.mult)
                    nc.vector.tensor_tensor(out=var[:], in0=var[:], in1=musq[:],
                                            op=Alu.subtract)
                    # sd = sqrt(var + eps); rinv = 1/sd; a = rinv*gamma; cc = bet - mu*a
                    sd = smallp.tile([H, 1], f32, tag="sd")
                    nc.scalar.activation(out=sd[:], in_=var[:], func=Act.Sqrt,
                                         bias=eps_t[:], scale=1.0)
                    rinv = smallp.tile([H, 1], f32, tag="rinv")
                    nc.vector.reciprocal(out=rinv[:], in_=sd[:])
                    a_t = smallp.tile([H, 1], f32, tag="a")
                    nc.vector.tensor_tensor(out=a_t[:], in0=rinv[:], in1=gam,
                                            op=Alu.mult)
                    cc_t = smallp.tile([H, 1], f32, tag="cc")
                    nc.vector.tensor_tensor(out=cc_t[:], in0=mu[:], in1=a_t[:],
                                            op=Alu.mult)
                    nc.vector.tensor_tensor(out=cc_t[:], in0=bet, in1=cc_t[:],
                                            op=Alu.subtract)

                    # ---- z = r*a + cc
                    z_t = actp.tile([H, nlocp], f32, tag="act")
                    for j in range(n512):
                        c0 = j * 512
                        cn = min(512, nlocp - c0)
                        nc.vector.tensor_scalar(out=z_t[:, c0:c0 + cn],
                                                in0=r_t[:, c0:c0 + cn],
                                                scalar1=a_t[:], scalar2=cc_t[:],
                                                op0=Alu.mult, op1=Alu.add)

                    # ---- outputs: z rows + pooling
                    ppool = psPool.tile([P, H], f32, tag="pool")
                    for t in range(nw):
                        c0 = t * P
                        pt_ps = ps128.tile([P, H], f32, tag="p128")
                        nc.tensor.transpose(out=pt_ps[:], in_=z_t[:, c0:c0 + P],
                                            identity=ident_t[:])
                        zr = zrowp.tile([P, H], f32, tag="zrow")
                        nc.scalar.copy(out=zr[:], in_=pt_ps[:])
                        nc.sync.dma_start(out=z_out[l, c0:c0 + P, :], in_=zr[:])
                        nc.tensor.matmul(out=ppool[:],
                                         lhsT=ptab_t[:, c0:c0 + P],
                                         rhs=zr[:], start=(t == 0),
                                         stop=(t == nw - 1))
                    gp = zrowp.tile([P, H], f32, tag="zrow")
                    nc.scalar.copy(out=gp[:], in_=ppool[:])
                    nc.sync.dma_start(out=g_out[l, :, :], in_=gp[:])

                    act_t = z_t

    nc.compile()
    return nc


def make_in_maps(cfg, x, W1s, W2s, b1s, b2s, gs, bs):
    """Build the per-core input maps. W1s[0] is [DIN,H]; others [H,H]."""
    vecs = np.zeros((cfg.H, 4 * cfg.L), np.float32)
    for l in range(cfg.L):
        vecs[:, 4 * l + 0] = b1s[l]
        vecs[:, 4 * l + 1] = b2s[l]
        vecs[:, 4 * l + 2] = gs[l]
        vecs[:, 4 * l + 3] = bs[l]
    iota = np.tile(np.arange(P, dtype=np.float32), (P, 1))
    ident = np.eye(cfg.H, dtype=np.float32)
    in_maps = []
    for c in range(NCORES):
        xt = np.zeros((cfg.DIN, cfg.nlocp), np.float32)
        xt[:, :cfg.nloc] = x[c * cfg.nloc:(c + 1) * cfg.nloc].T
        m = {
            "xt": xt,
            "vecs": vecs,
            "gidxA": cfg.gidxA[c],
            "gidxB": cfg.gidxB[c] if cfg.nchB > 0 else
                     np.zeros((P, P // 16), np.int16),
            "dtab": cfg.dtab[c],
            "ptab": cfg.ptab[c],
            "iota": iota,
            "ident": ident,
        }
        for l in range(cfg.L):
            m[f"w1_{l}"] = np.ascontiguousarray(W1s[l].astype(np.float32))
            m[f"w2_{l}"] = np.ascontiguousarray(W2s[l].astype(np.float32))
        in_maps.append(m)
    return in_maps


def assemble_outputs(cfg, results):
    """results: list of {"z_out": [L,nlocp,H], "g_out": [L,128,H]} per core."""
    L, H, N, G = cfg.L, cfg.H, cfg.N, cfg.G
    z_cat = np.zeros((N, L * H), np.float32)
    g_cat = np.zeros((G, L * H), np.float32)
    for c in range(NCORES):
        z = np.asarray(results[c]["z_out"]).reshape(L, cfg.nlocp, H)
        for l in range(L):
            z_cat[c * cfg.nloc:(c + 1) * cfg.nloc, l * H:(l + 1) * H] = \
                z[l, :cfg.nloc, :]
        g = np.asarray(results[c]["g_out"]).reshape(L, 128, H)
        lo = cfg.g_lo[c]
        nsl = min(128, G - lo)
        for l in range(L):
            g_cat[lo:lo + nsl, l * H:(l + 1) * H] += g[l, :nsl, :]
    return z_cat, g_cat


def kernel(x, edge_index, batch, W1_0, b1_0, W2_0, b2_0, bn_g0, bn_b0,
           W1_r, b1_r, W2_r, b2_r, bn_gr, bn_br):
    from concourse.bass_utils import run_bass_kernel_spmd

    x = np.asarray(x, np.float32)
    edge_index = np.asarray(edge_index)
    batch = np.asarray(batch)
    N, DIN = x.shape
    E = edge_index.shape[1]
    G = int(batch.max()) + 1
    L = np.asarray(W1_r).shape[0] + 1
    H = np.asarray(W1_0).shape[1]

    cfg = make_config(N, E, max(G, 512), DIN, H, L, edge_index, batch)
    W1s = [np.asarray(W1_0)] + [np.asarray(W1_r)[i] for i in range(L - 1)]
    W2s = [np.asarray(W2_0)] + [np.asarray(W2_r)[i] for i in range(L - 1)]
    b1s = [np.asarray(b1_0)] + [np.asarray(b1_r)[i] for i in range(L - 1)]
    b2s = [np.asarray(b2_0)] + [np.asarray(b2_r)[i] for i in range(L - 1)]
    gs = [np.asarray(bn_g0)] + [np.asarray(bn_gr)[i] for i in range(L - 1)]
    bs = [np.asarray(bn_b0)] + [np.asarray(bn_br)[i] for i in range(L - 1)]

    nc = build_program(cfg)
    in_maps = make_in_maps(cfg, x, W1s, W2s, b1s, b2s, gs, bs)
    r = run_bass_kernel_spmd(nc, in_maps, list(range(NCORES)))
    return assemble_outputs(cfg, [r.results[c] for c in range(NCORES)])


# revision 14
# speedup vs baseline: 226.5306x; 226.5306x over previous
"""GIN message-passing network on 8 Trainium2 NeuronCores.

Strategy (data parallel over nodes/edges, dst-sharded):
 - Nodes are split into 8 contiguous chunks (one per core), padded to a
   multiple of 128.  Edges are assigned to the core that owns their dst.
 - Per layer, using linearity of the aggregation:
       h = MLP(z + A z) ;  u = z @ W1 ;  m = relu(u + A u + b1)
   so each core computes its local u chunk, all-gathers u (row-major
   table in DRAM), then gathers u[src] for its edges with dma_gather and
   segment-sums them into per-128-node windows with one small matmul per
   128-edge chunk (selection matrix built on the vector engine from a
   precomputed window-local dst table).  BatchNorm statistics are
   all-reduced ([64,2] sums).  Activations live feature-on-partition.
 - Gather indices are int16, so the u table is addressed through two
   base offsets (rows < 32768 and >= 32768); edges are split into two
   streams accordingly, each with its own chunk schedule per window.
"""

import math
import numpy as np

NCORES = 8
P = 128


def _ceil_div(a, b):
    return -(-a // b)


def _pack_idx(idx):
    """Wrap an int array into the [128, n/16] int16 layout dma_gather wants
    (index i at [i % 16, i // 16], replicated across the 8 Q7 stripes)."""
    n = len(idx)
    assert n % 16 == 0
    arr = np.asarray(idx, dtype=np.int16).reshape(-1, 16).T.copy()  # [16, n/16]
    return np.tile(arr, (8, 1))


class Cfg:
    pass


def make_config(N, E, G, DIN, H, L, edge_index, batch, split=32768,
                call_chunks=32, reps=1, gbufs=3, ws=128):
    cfg = Cfg()
    cfg.N, cfg.E, cfg.G, cfg.DIN, cfg.H, cfg.L = N, E, G, DIN, H, L
    assert N % NCORES == 0
    cfg.nloc = N // NCORES
    cfg.nlocp = _ceil_div(cfg.nloc, P) * P
    while cfg.nlocp % ws != 0:
        ws -= 128
    cfg.ws = ws
    cfg.nw = cfg.nlocp // ws
    cfg.nwt = cfg.nlocp // P
    cfg.ntab = NCORES * cfg.nlocp          # padded u-table rows
    cfg.split = split
    assert cfg.split % 2 == 0 and cfg.split <= 32768
    assert cfg.ntab - cfg.split < 32768 and cfg.split < cfg.ntab
    cfg.call_chunks = call_chunks
    cfg.reps = reps
    cfg.gbufs = gbufs
    cfg.eps = 1e-5
    cfg.n512 = _ceil_div(cfg.nlocp, 512)
    assert cfg.nlocp % 512 == 0 or cfg.nlocp % 128 == 0

    src = np.asarray(edge_index[0], dtype=np.int64)
    dst = np.asarray(edge_index[1], dtype=np.int64)
    owner = dst // cfg.nloc
    ld = dst - owner * cfg.nloc
    src_pad = (src // cfg.nloc) * cfg.nlocp + (src % cfg.nloc)

    # per-core edge lists sorted by local dst
    per_core = []
    for c in range(NCORES):
        m = owner == c
        sp, l = src_pad[m], ld[m]
        o = np.argsort(l, kind="stable")
        per_core.append((sp[o], l[o]))

    # chunk schedule: CA[w], CB[w] = shared (max over cores) chunk counts
    CA = np.zeros(cfg.nw, np.int64)
    CB = np.zeros(cfg.nw, np.int64)
    counts = []  # per core: (nA[w], nB[w])
    for c in range(NCORES):
        sp, l = per_core[c]
        w = l // cfg.ws
        isb = sp >= cfg.split
        nA = np.bincount(w[~isb], minlength=cfg.nw)
        nB = np.bincount(w[isb], minlength=cfg.nw)
        counts.append((nA, nB))
        CA = np.maximum(CA, _ceil_div(nA, P))
        CB = np.maximum(CB, _ceil_div(nB, P))
    # ensure every window has at least one chunk so psum gets initialized
    for w in range(cfg.nw):
        if CA[w] == 0 and CB[w] == 0:
            CA[w] = 1
    cfg.CA, cfg.CB = CA, CB
    cfg.nchA, cfg.nchB = int(CA.sum()), int(CB.sum())
    cfg.nch = cfg.nchA + cfg.nchB
    # column index of first chunk of each window in each stream
    cfg.acol = np.concatenate([[0], np.cumsum(CA)])[:-1]
    cfg.bcol = np.concatenate([[0], np.cumsum(CB)])[:-1]

    # per-core data: gather index streams + d-table
    cfg.gidxA, cfg.gidxB, cfg.dtab = [], [], []
    for c in range(NCORES):
        sp, l = per_core[c]
        w = l // cfg.ws
        isb = sp >= cfg.split
        dloc = (l - w * cfg.ws).astype(np.float32)
        iA = np.zeros(cfg.nchA * P, np.int64)
        iB = np.zeros(cfg.nchB * P, np.int64)
        dt = np.full((P, cfg.nch), -1.0, np.float32)
        for wi in range(cfg.nw):
            mA = (w == wi) & ~isb
            mB = (w == wi) & isb
            for (mask, C, col0, stream, base) in (
                (mA, CA, cfg.acol, iA, 0),
                (mB, CB, cfg.bcol, iB, cfg.split),
            ):
                vals = sp[mask] - base
                ds = dloc[mask]
                n = len(vals)
                off = col0[wi] * P
                stream[off:off + n] = vals
                # d-table column block for this window's chunks
                colbase = col0[wi] + (0 if base == 0 else cfg.nchA)
                nchunks = C[wi]
                dcol = np.full(nchunks * P, -1.0, np.float32)
                dcol[:n] = ds
                dt[:, colbase:colbase + nchunks] = dcol.reshape(nchunks, P).T
        cfg.gidxA.append(_pack_idx(iA))
        cfg.gidxB.append(_pack_idx(iB))
        cfg.dtab.append(dt)

    # (1 + in-degree) row per core, padded
    cfg.degrow = []
    degs = np.bincount(dst, minlength=N).astype(np.float32)
    for c in range(NCORES):
        dr = np.zeros((1, cfg.nlocp), np.float32)
        dr[0, :cfg.nloc] = 1.0 + degs[c * cfg.nloc:(c + 1) * cfg.nloc]
        cfg.degrow.append(dr)
    assert cfg.nlocp >= cfg.nloc + 2, "need 2 pad rows for stats"

    # pooling: slots per core (fixed across cores)
    b = np.asarray(batch, dtype=np.int64)
    cfg.g_lo = []
    spans = []
    for c in range(NCORES):
        bc = b[c * cfg.nloc:(c + 1) * cfg.nloc]
        cfg.g_lo.append(int(bc[0]))
        spans.append(int(bc[-1]) - int(bc[0]) + 1)
    cfg.nslot = _ceil_div(max(spans), P) * P
    assert cfg.nslot <= 128, f"graph span {max(spans)} exceeds 128 slots"
    cfg.nslot = 128
    cfg.ptab = []
    for c in range(NCORES):
        bc = b[c * cfg.nloc:(c + 1) * cfg.nloc]
        pt = np.zeros((P, cfg.nwt * P), np.float32)
        rows = np.arange(cfg.nloc)
        slot = bc - cfg.g_lo[c]
        pt[rows % P, (rows // P) * P + slot] = 1.0
        cfg.ptab.append(pt)
    return cfg


def build_program(cfg, skip=()):
    import concourse.bacc as bacc
    import concourse.tile as tile
    from concourse import mybir

    f32 = mybir.dt.float32
    i16 = mybir.dt.int16
    Alu = mybir.AluOpType
    Act = mybir.ActivationFunctionType

    H, DIN, L = cfg.H, cfg.DIN, cfg.L
    nlocp, nw, n512 = cfg.nlocp, cfg.nw, cfg.n512
    nloc, ws, nwt = cfg.nloc, cfg.ws, cfg.nwt

    nc = bacc.Bacc()

    # ---- parameters -------------------------------------------------
    xt_p = nc.declare_dram_parameter("xt", [DIN, nlocp], f32, isOutput=False)
    w1_p = [nc.declare_dram_parameter(f"w1_{l}", [DIN if l == 0 else H, H], f32,
                                      isOutput=False) for l in range(L)]
    w2_p = [nc.declare_dram_parameter(f"w2_{l}", [H, H], f32, isOutput=False)
            for l in range(L)]
    vecs_p = nc.declare_dram_parameter("vecs", [H, 4 * L], f32, isOutput=False)
    idxA_p = nc.declare_dram_parameter("gidxA", [P, cfg.nchA * P // 16], i16,
                                       isOutput=False)
    idxB_p = nc.declare_dram_parameter("gidxB", [P, max(cfg.nchB, 1) * P // 16], i16,
                                       isOutput=False)
    dtab_p = nc.declare_dram_parameter("dtab", [P, cfg.nch], f32, isOutput=False)
    ptab_p = nc.declare_dram_parameter("ptab", [P, nwt * P], f32, isOutput=False)
    iota_p = nc.declare_dram_parameter("iota", [P, cfg.ws], f32, isOutput=False)
    ident_p = nc.declare_dram_parameter("ident", [H, H], f32, isOutput=False)
    z_out = nc.declare_dram_parameter("z_out", [L, nlocp, H], f32, isOutput=True)
    g_out = nc.declare_dram_parameter("g_out", [L, P, H], f32, isOutput=True)

    # ---- internal DRAM (collectives) --------------------------------
    ag_in = nc.dram_tensor("ag_in", [nlocp, H], f32)
    u_full = nc.dram_tensor("u_full", [cfg.ntab, H], f32, addr_space="Shared")
    ar_in = nc.dram_tensor("ar_in", [H, 2], f32)
    ar_out = nc.dram_tensor("ar_out", [H, 2], f32, addr_space="Shared")
    rg = [list(range(NCORES))]

    # gather calls: (start_chunk, n_chunks) per call, per stream
    def calls_of(nch):
        out = []
        s = 0
        while s < nch:
            n = min(cfg.call_chunks, nch - s)
            out.append((s, n))
            s += n
        return out

    callsA = calls_of(cfg.nchA)
    callsB = calls_of(cfg.nchB)

    # map chunk -> (call_index, slot)
    def chunk_map(calls):
        m = {}
        for ci, (s, n) in enumerate(calls):
            for k in range(n):
                m[s + k] = (ci, k)
        return m

    mapA = chunk_map(callsA)
    mapB = chunk_map(callsB)

    # stats chunks covering exactly nloc columns
    stat_chunks = []
    off = 0
    while off < nloc:
        n = min(512, nloc - off)
        stat_chunks.append((off, n))
        off += n
    nstat = len(stat_chunks)

    with tile.TileContext(nc) as tc:
        with (
            tc.tile_pool(name="const", bufs=1) as constp,
            tc.tile_pool(name="act", bufs=1) as actp,
            tc.tile_pool(name="uown", bufs=1) as uownp,
            tc.tile_pool(name="ga", bufs=cfg.gbufs) as gap,
            tc.tile_pool(name="gb", bufs=max(2, cfg.gbufs // 2)) as gbp,
            tc.tile_pool(name="sp", bufs=4) as spp,
            tc.tile_pool(name="small", bufs=4) as smallp,
            tc.tile_pool(name="urow", bufs=3) as urowp,
            tc.tile_pool(name="zrow", bufs=3) as zrowp,
            tc.tile_pool(name="psA", bufs=2, space="PSUM") as psA,
            tc.tile_pool(name="psB", bufs=2, space="PSUM") as psB,
            tc.tile_pool(name="psUH", bufs=2, space="PSUM") as psUH,
            tc.tile_pool(name="ps128", bufs=1, space="PSUM") as ps128,
            tc.tile_pool(name="psPool", bufs=1, space="PSUM") as psPool,
        ):
            # resident constants
            idxA_t = constp.tile([P, cfg.nchA * P // 16], i16)
            nc.sync.dma_start(out=idxA_t[:], in_=idxA_p[:, :])
            idxB_t = constp.tile([P, max(cfg.nchB, 1) * P // 16], i16)
            nc.sync.dma_start(out=idxB_t[:], in_=idxB_p[:, :])
            dtab_t = constp.tile([P, cfg.nch], f32)
            nc.sync.dma_start(out=dtab_t[:], in_=dtab_p[:, :])
            ptab_t = constp.tile([P, nwt * P], f32)
            nc.sync.dma_start(out=ptab_t[:], in_=ptab_p[:, :])
            iota_t = constp.tile([P, cfg.ws], f32)
            nc.sync.dma_start(out=iota_t[:], in_=iota_p[:, :])
            ident_t = constp.tile([H, H], f32)
            nc.sync.dma_start(out=ident_t[:], in_=ident_p[:, :])
            w1_t = []
            w2_t = []
            for l in range(L):
                d = DIN if l == 0 else H
                t = constp.tile([d, H], f32, tag=f"w1_{l}")
                nc.sync.dma_start(out=t[:], in_=w1_p[l][:, :])
                w1_t.append(t)
                t = constp.tile([H, H], f32, tag=f"w2_{l}")
                nc.sync.dma_start(out=t[:], in_=w2_p[l][:, :])
                w2_t.append(t)
            vecs_t = constp.tile([H, 4 * L], f32)
            nc.sync.dma_start(out=vecs_t[:], in_=vecs_p[:, :])
            eps_t = constp.tile([H, 1], f32)
            nc.vector.memset(eps_t[:], cfg.eps)

            for rep in range(cfg.reps):
                xt_t = actp.tile([DIN, nlocp], f32, tag="act")
                nc.sync.dma_start(out=xt_t[:], in_=xt_p[:, :])
                act_t = xt_t

                for l in range(L):
                    d_in = DIN if l == 0 else H
                    b1 = vecs_t[:, 4 * l + 0:4 * l + 1]
                    b2 = vecs_t[:, 4 * l + 1:4 * l + 2]
                    gam = vecs_t[:, 4 * l + 2:4 * l + 3]
                    bet = vecs_t[:, 4 * l + 3:4 * l + 4]

                    # ---- u = act @ W1 : transposed copy + row-major to DRAM
                    uown_t = uownp.tile([H, nlocp], f32, tag="uown")
                    for j in range(n512):
                        c0 = j * 512
                        cn = min(512, nlocp - c0)
                        pu = psUH.tile([H, 512], f32, tag="psuh")
                        nc.tensor.matmul(out=pu[:, :cn], lhsT=w1_t[l][:],
                                         rhs=act_t[:, c0:c0 + cn],
                                         start=True, stop=True)
                        nc.scalar.copy(out=uown_t[:, c0:c0 + cn], in_=pu[:, :cn])
                    for t in range(nwt):
                        c0 = t * P
                        pur = ps128.tile([P, H], f32, tag="p128")
                        nc.tensor.matmul(out=pur[:], lhsT=act_t[:, c0:c0 + P],
                                         rhs=w1_t[l][:], start=True, stop=True)
                        ur = urowp.tile([P, H], f32, tag="urow")
                        nc.scalar.copy(out=ur[:], in_=pur[:])
                        nc.sync.dma_start(out=ag_in[c0:c0 + P, :], in_=ur[:])

                    # ---- all-gather u
                    if "coll" not in skip:
                        nc.gpsimd.collective_compute(
                            "AllGather", mybir.AluOpType.bypass,
                            ins=[ag_in[:, :]], outs=[u_full[:, :]],
                            replica_groups=rg,
                        )
                    else:
                        nc.sync.dma_start(out=u_full[0:nlocp, :], in_=ag_in[:, :])

                    # ---- gathers
                    gtilesA = []
                    for (s, n) in callsA:
                        g = gap.tile([P, cfg.call_chunks, H], f32, tag="ga")
                        if "gather" in skip:
                            nc.vector.memset(g[:, :n, :], 0.1)
                            gtilesA.append(g)
                            continue
                        nc.gpsimd.dma_gather(
                            out_ap=g[:, :n, :],
                            in_ap=u_full[0:cfg.split, :],
                            idxs_ap=idxA_t[:, s * 8:(s + n) * 8],
                            num_idxs=n * P, num_idxs_reg=n * P,
                            elem_size=H, single_packet=False,
                        )
                        gtilesA.append(g)
                    gtilesB = []
                    for (s, n) in callsB:
                        g = gbp.tile([P, cfg.call_chunks, H], f32, tag="gb")
                        if "gather" in skip:
                            nc.vector.memset(g[:, :n, :], 0.1)
                            gtilesB.append(g)
                            continue
                        nc.gpsimd.dma_gather(
                            out_ap=g[:, :n, :],
                            in_ap=u_full[cfg.split:cfg.ntab, :],
                            idxs_ap=idxB_t[:, s * 8:(s + n) * 8],
                            num_idxs=n * P, num_idxs_reg=n * P,
                            elem_size=H, single_packet=False,
                        )
                        gtilesB.append(g)

                    # ---- windows: selection-matrix matmuls into PSUM,
                    # accumulated in place into uown (becomes m_pre -> m -> r)
                    mpre_t = uown_t
                    for w in range(nw) if "windows" not in skip else []:
                        pa = None
                        if cfg.CA[w] > 0:
                            pa = psA.tile([H, ws], f32, tag="psa")
                            for k in range(cfg.CA[w]):
                                ch = int(cfg.acol[w]) + k
                                ci, slot = mapA[ch]
                                st = spp.tile([P, ws], f32, tag="s")
                                nc.vector.tensor_scalar(
                                    out=st[:], in0=iota_t[:],
                                    scalar1=dtab_t[:, ch:ch + 1],
                                    scalar2=None, op0=Alu.is_equal)
                                nc.tensor.matmul(
                                    out=pa[:], lhsT=gtilesA[ci][:, slot, :],
                                    rhs=st[:], start=(k == 0),
                                    stop=(k == cfg.CA[w] - 1))
                        pb = None
                        if cfg.CB[w] > 0:
                            pb = psB.tile([H, ws], f32, tag="psb")
                            for k in range(cfg.CB[w]):
                                ch = int(cfg.bcol[w]) + k
                                ci, slot = mapB[ch]
                                st = spp.tile([P, ws], f32, tag="s")
                                nc.vector.tensor_scalar(
                                    out=st[:], in0=iota_t[:],
                                    scalar1=dtab_t[:, cfg.nchA + ch:cfg.nchA + ch + 1],
                                    scalar2=None, op0=Alu.is_equal)
                                nc.tensor.matmul(
                                    out=pb[:], lhsT=gtilesB[ci][:, slot, :],
                                    rhs=st[:], start=(k == 0),
                                    stop=(k == cfg.CB[w] - 1))
                        c0 = w * ws
                        first = pa if pa is not None else pb
                        nc.vector.tensor_add(out=mpre_t[:, c0:c0 + ws],
                                             in0=first[:],
                                             in1=uown_t[:, c0:c0 + ws])
                        if pa is not None and pb is not None:
                            nc.vector.tensor_add(out=mpre_t[:, c0:c0 + ws],
                                                 in0=mpre_t[:, c0:c0 + ws],
                                                 in1=pb[:])

                    # ---- m = relu(mpre + b1) in place ; h = m @ W2 ;
                    # r = relu(h + b2) overwrites m tile by tile
                    m_t = mpre_t
                    for j in range(n512):
                        c0 = j * 512
                        cn = min(512, nlocp - c0)
                        nc.scalar.activation(out=m_t[:, c0:c0 + cn],
                                             in_=m_t[:, c0:c0 + cn],
                                             func=Act.Relu, bias=b1, scale=1.0)
                    r_t = m_t
                    for j in range(n512):
                        c0 = j * 512
                        cn = min(512, nlocp - c0)
                        ph = psUH.tile([H, 512], f32, tag="psuh")
                        nc.tensor.matmul(out=ph[:, :cn], lhsT=w2_t[l][:],
                                         rhs=m_t[:, c0:c0 + cn],
                                         start=True, stop=True)
                        nc.scalar.activation(out=r_t[:, c0:c0 + cn],
                                             in_=ph[:, :cn],
                                             func=Act.Relu, bias=b2, scale=1.0)

                    # ---- batchnorm stats over the real nloc columns
                    stats_t = smallp.tile([H, nstat, 6], f32, tag="stats")
                    for j, (o, n) in enumerate(stat_chunks):
                        nc.vector.bn_stats(out=stats_t[:, j, :],
                                           in_=r_t[:, o:o + n])
                    mv = smallp.tile([H, 2], f32, tag="mv")
                    nc.vector.bn_aggr(out=mv[:], in_=stats_t[:])
                    # sums: sx = mean*nloc ; sxx = (var + mean^2)*nloc
                    sums = smallp.tile([H, 2], f32, tag="sums")
                    nc.vector.tensor_scalar(out=sums[:, 0:1], in0=mv[:, 0:1],
                                            scalar1=float(nloc), scalar2=None,
                                            op0=Alu.mult)
                    sq = smallp.tile([H, 1], f32, tag="sq")
                    nc.vector.tensor_tensor(out=sq[:], in0=mv[:, 0:1],
                                            in1=mv[:, 0:1], op=Alu.mult)
                    nc.vector.tensor_add(out=sq[:], in0=sq[:], in1=mv[:, 1:2])
                    nc.vector.tensor_scalar(out=sums[:, 1:2], in0=sq[:],
                                            scalar1=float(nloc), scalar2=None,
                                            op0=Alu.mult)
                    nc.gpsimd.dma_start(out=ar_in[:, :], in_=sums[:])
                    if "coll" not in skip:
                        nc.gpsimd.collective_compute(
                            "AllReduce", mybir.AluOpType.add,
                            ins=[ar_in[:, :]], outs=[ar_out[:, :]],
                            replica_groups=rg,
                        )
                    else:
                        nc.gpsimd.dma_start(out=ar_out[:, :], in_=ar_in[:, :])
                    gs = smallp.tile([H, 2], f32, tag="gs")
                    nc.gpsimd.dma_start(out=gs[:], in_=ar_out[:, :])
                    # mu = gs[:,0]/N ; ex2 = gs[:,1]/N ; var = ex2 - mu^2
                    mu = smallp.tile([H, 1], f32, tag="mu")
                    nc.vector.tensor_scalar(out=mu[:], in0=gs[:, 0:1],
                                            scalar1=1.0 / cfg.N, scalar2=None,
                                            op0=Alu.mult)
                    var = smallp.tile([H, 1], f32, tag="var")
                    nc.vector.tensor_scalar(out=var[:], in0=gs[:, 1:2],
                                            scalar1=1.0 / cfg.N, scalar2=None,
                                            op0=Alu.mult)
                    musq = smallp.tile([H, 1], f32, tag="musq")
                    nc.vector.tensor_tensor(out=musq[:], in0=mu[:], in1=mu[:],
                                            op=Alu.mult)
                    nc.vector.tensor_tensor(out=var[:], in0=var[:], in1=musq[:],
                                            op=Alu.subtract)
                    # sd = sqrt(var + eps); rinv = 1/sd; a = rinv*gamma; cc = bet - mu*a
                    sd = smallp.tile([H, 1], f32, tag="sd")
                    nc.scalar.activation(out=sd[:], in_=var[:], func=Act.Sqrt,
                                         bias=eps_t[:], scale=1.0)
                    rinv = smallp.tile([H, 1], f32, tag="rinv")
                    nc.vector.reciprocal(out=rinv[:], in_=sd[:])
                    a_t = smallp.tile([H, 1], f32, tag="a")
                    nc.vector.tensor_tensor(out=a_t[:], in0=rinv[:], in1=gam,
                                            op=Alu.mult)
                    cc_t = smallp.tile([H, 1], f32, tag="cc")
                    nc.vector.tensor_tensor(out=cc_t[:], in0=mu[:], in1=a_t[:],
                                            op=Alu.mult)
                    nc.vector.tensor_tensor(out=cc_t[:], in0=bet, in1=cc_t[:],
                                            op=Alu.subtract)

                    # ---- z = r*a + cc
                    z_t = actp.tile([H, nlocp], f32, tag="act")
                    for j in range(n512):
                        c0 = j * 512
                        cn = min(512, nlocp - c0)
                        nc.vector.tensor_scalar(out=z_t[:, c0:c0 + cn],
                                                in0=r_t[:, c0:c0 + cn],
                                                scalar1=a_t[:], scalar2=cc_t[:],
                                                op0=Alu.mult, op1=Alu.add)

                    # ---- outputs: z rows + pooling
                    ppool = psPool.tile([P, H], f32, tag="pool")
                    for t in range(nwt) if "outs" not in skip else []:
                        c0 = t * P
                        pt_ps = ps128.tile([P, H], f32, tag="p128")
                        nc.tensor.transpose(out=pt_ps[:], in_=z_t[:, c0:c0 + P],
                                            identity=ident_t[:])
                        zr = zrowp.tile([P, H], f32, tag="zrow")
                        nc.scalar.copy(out=zr[:], in_=pt_ps[:])
                        nc.sync.dma_start(out=z_out[l, c0:c0 + P, :], in_=zr[:])
                        nc.tensor.matmul(out=ppool[:],
                                         lhsT=ptab_t[:, c0:c0 + P],
                                         rhs=zr[:], start=(t == 0),
                                         stop=(t == nwt - 1))
                    if "outs" not in skip:
                        gp = zrowp.tile([P, H], f32, tag="zrow")
                        nc.scalar.copy(out=gp[:], in_=ppool[:])
                        nc.sync.dma_start(out=g_out[l, :, :], in_=gp[:])

                    act_t = z_t

    nc.compile()
    return nc


def make_in_maps(cfg, x, W1s, W2s, b1s, b2s, gs, bs):
    """Build the per-core input maps. W1s[0] is [DIN,H]; others [H,H]."""
    vecs = np.zeros((cfg.H, 4 * cfg.L), np.float32)
    for l in range(cfg.L):
        vecs[:, 4 * l + 0] = b1s[l]
        vecs[:, 4 * l + 1] = b2s[l]
        vecs[:, 4 * l + 2] = gs[l]
        vecs[:, 4 * l + 3] = bs[l]
    iota = np.tile(np.arange(cfg.ws, dtype=np.float32), (P, 1))
    ident = np.eye(cfg.H, dtype=np.float32)
    in_maps = []
    for c in range(NCORES):
        xt = np.zeros((cfg.DIN, cfg.nlocp), np.float32)
        xt[:, :cfg.nloc] = x[c * cfg.nloc:(c + 1) * cfg.nloc].T
        m = {
            "xt": xt,
            "vecs": vecs,
            "gidxA": cfg.gidxA[c],
            "gidxB": cfg.gidxB[c] if cfg.nchB > 0 else
                     np.zeros((P, P // 16), np.int16),
            "dtab": cfg.dtab[c],
            "ptab": cfg.ptab[c],
            "iota": iota,
            "ident": ident,
            "degrow": cfg.degrow[c],
        }
        for l in range(cfg.L):
            m[f"w1_{l}"] = np.ascontiguousarray(W1s[l].astype(np.float32))
            m[f"w2_{l}"] = np.ascontiguousarray(W2s[l].astype(np.float32))
        in_maps.append(m)
    return in_maps


def assemble_outputs(cfg, results):
    """results: list of {"z_out": [L,nlocp,H], "g_out": [L,128,H]} per core."""
    L, H, N, G = cfg.L, cfg.H, cfg.N, cfg.G
    z_cat = np.zeros((N, L * H), np.float32)
    g_cat = np.zeros((G, L * H), np.float32)
    for c in range(NCORES):
        z = np.asarray(results[c]["z_out"]).reshape(L, cfg.nlocp, H)
        for l in range(L):
            z_cat[c * cfg.nloc:(c + 1) * cfg.nloc, l * H:(l + 1) * H] = \
                z[l, :cfg.nloc, :]
        g = np.asarray(results[c]["g_out"]).reshape(L, 128, H)
        lo = cfg.g_lo[c]
        nsl = min(128, G - lo)
        for l in range(L):
            g_cat[lo:lo + nsl, l * H:(l + 1) * H] += g[l, :nsl, :]
    return z_cat, g_cat


def kernel(x, edge_index, batch, W1_0, b1_0, W2_0, b2_0, bn_g0, bn_b0,
           W1_r, b1_r, W2_r, b2_r, bn_gr, bn_br):
    from concourse.bass_utils import run_bass_kernel_spmd

    x = np.asarray(x, np.float32)
    edge_index = np.asarray(edge_index)
    batch = np.asarray(batch)
    N, DIN = x.shape
    E = edge_index.shape[1]
    G = int(batch.max()) + 1
    L = np.asarray(W1_r).shape[0] + 1
    H = np.asarray(W1_0).shape[1]

    cfg = make_config(N, E, max(G, 512), DIN, H, L, edge_index, batch)
    W1s = [np.asarray(W1_0)] + [np.asarray(W1_r)[i] for i in range(L - 1)]
    W2s = [np.asarray(W2_0)] + [np.asarray(W2_r)[i] for i in range(L - 1)]
    b1s = [np.asarray(b1_0)] + [np.asarray(b1_r)[i] for i in range(L - 1)]
    b2s = [np.asarray(b2_0)] + [np.asarray(b2_r)[i] for i in range(L - 1)]
    gs = [np.asarray(bn_g0)] + [np.asarray(bn_gr)[i] for i in range(L - 1)]
    bs = [np.asarray(bn_b0)] + [np.asarray(bn_br)[i] for i in range(L - 1)]

    nc = build_program(cfg)
    in_maps = make_in_maps(cfg, x, W1s, W2s, b1s, b2s, gs, bs)
    r = run_bass_kernel_spmd(nc, in_maps, list(range(NCORES)))
    return assemble_outputs(cfg, [r.results[c] for c in range(NCORES)])
